# revision 1
# baseline (speedup 1.0000x reference)
"""Trainium2 Bass kernel for LocalGlobalSelfAttention (v3: fp8 DoubleRow AV).

Sharding: 8 cores = 4 batches x 2 sequence-halves (no collectives).
Each core computes, for its (batch b, half h):
  - global attention: queries = its half (SH rows), keys/values = full seq
  - local windowed attention: fully contained in its half
  - output projections (g+l accumulated in PSUM) + bias + residual + layernorm

Structure: one software-pipelined loop over head-pairs; projections for
head-pair hp interleave with attention for hp-1 so ScalarE exp overlaps PE
matmuls (keeps PE dense/warm). V projections run in groups of 4 head-pairs
(N=512 moving dim). V and the exp'd scores are quantized to fp8e4; the AV
matmul uses DoubleRow perf mode (contraction 256 = two s-tiles per
instruction), which also halves the per-matmul LDWEIGHTS overhead. Softmax
rowsums ride the AV via a ones column; normalization = raw-evac to SBUF +
reciprocal_approx_fast + gpsimd broadcast + one DVE multiply.
"""

import numpy as np
import ml_dtypes
from contextlib import ExitStack

BF16 = ml_dtypes.bfloat16
FP8 = ml_dtypes.float8_e4m3

FULL_CFG = dict(S=2048, D=1024, H=16, K=64, NW=8)
N_CORES = 8
LN_EPS = 1e-3


def _chunks(total, size):
    return [(o, min(size, total - o)) for o in range(0, total, size)]


def build_nc(cfg=None, n_dev=N_CORES, av_mode="dr_fp8", dbg=False):
    """Build + compile the per-core Bass program (SPMD, same on all cores)."""
    import concourse.bass as bass
    import concourse.tile as tile
    import concourse.mybir as mybir
    from concourse import bacc

    cfg = dict(cfg or FULL_CFG)
    S, D, H, K, NW = cfg["S"], cfg["D"], cfg["H"], cfg["K"], cfg["NW"]
    HK = H * K
    SH = S // 2          # per-core query rows (half the sequence)
    WIN = S // NW        # local attention window
    NWH = SH // WIN      # windows in this core's half
    assert K == 64 and D % 128 == 0 and HK % 128 == 0

    ND = D // 128        # d-tiles
    NHK = HK // 128      # head-pair tiles (2 heads each)
    NST = S // 128       # s-tiles (full seq)
    NQT = SH // 128      # q-tiles (half seq)
    QC = 512             # query chunk for AV / o accumulation
    NQC = SH // QC
    NGRP = 2             # v-projection groups (4 head-pairs each)
    GHP = NHK // NGRP

    f32 = mybir.dt.float32
    bf16 = mybir.dt.bfloat16
    fp8 = mybir.dt.float8e4 if av_mode == "dr_fp8" else mybir.dt.bfloat16
    DR = mybir.MatmulPerfMode.DoubleRow
    Exp = mybir.ActivationFunctionType.Exp
    Square = mybir.ActivationFunctionType.Square
    Sqrt = mybir.ActivationFunctionType.Sqrt
    add_op = mybir.AluOpType.add
    mult_op = mybir.AluOpType.mult
    sub_op = mybir.AluOpType.subtract
    AxX = mybir.AxisListType.X

    nc = bacc.Bacc("TRN2", target_bir_lowering=False, debug=False,
                   num_devices=n_dev)

    # ---- DRAM parameters -------------------------------------------------
    xT_d = nc.dram_tensor("xT", [D, S], fp8, kind="ExternalInput")
    xq_d = nc.dram_tensor("xq", [SH, D], f32, kind="ExternalInput")
    # kq weights pre-shuffled host-side to [NHK, ND, 128, 128]
    w_d = {}
    for nm in ("wq_g", "wk_g", "wq_l", "wk_l"):
        w_d[nm] = nc.dram_tensor(nm, [NHK, ND, 128, 128], fp8,
                                 kind="ExternalInput")
    # v weights grouped: [NGRP, ND, 128, 4*128]
    wv_d = {}
    for nm in ("wv_g", "wv_l"):
        wv_d[nm] = nc.dram_tensor(nm, [NGRP, ND, 128, GHP * 128], fp8,
                                  kind="ExternalInput")
    wo_g_d = nc.dram_tensor("wo_g", [HK, D], bf16, kind="ExternalInput")
    wo_l_d = nc.dram_tensor("wo_l", [HK, D], bf16, kind="ExternalInput")
    bcol_d = {}
    for nm in ("bq_g", "bk_g", "bq_l", "bk_l"):
        bcol_d[nm] = nc.dram_tensor(nm, [NHK, 128], f32, kind="ExternalInput")
    bv_g_d = nc.dram_tensor("bv_g", [1, HK], bf16, kind="ExternalInput")
    bv_l_d = nc.dram_tensor("bv_l", [1, HK], bf16, kind="ExternalInput")
    bo_d = nc.dram_tensor("bo", [1, D], bf16, kind="ExternalInput")
    gamma_d = nc.dram_tensor("gamma", [1, D], bf16, kind="ExternalInput")
    beta_d = nc.dram_tensor("beta", [1, D], bf16, kind="ExternalInput")
    out_d = nc.dram_tensor("out", [SH, D], f32, kind="ExternalOutput")
    if dbg:
        dbg_d = {
            "dbg_kg": nc.dram_tensor("dbg_kg", [128, S], mybir.dt.bfloat16,
                                     kind="ExternalOutput"),
            "dbg_qg": nc.dram_tensor("dbg_qg", [128, SH], mybir.dt.bfloat16,
                                     kind="ExternalOutput"),
            "dbg_vg": nc.dram_tensor("dbg_vg", [128, 2 * 2 * 80],
                                     mybir.dt.float8e4 if av_mode == "dr_fp8"
                                     else mybir.dt.bfloat16,
                                     kind="ExternalOutput"),
            "dbg_og0": nc.dram_tensor("dbg_og0", [128, SH],
                                      mybir.dt.bfloat16,
                                      kind="ExternalOutput"),
            "dbg_og7": nc.dram_tensor("dbg_og7", [128, SH],
                                      mybir.dt.bfloat16,
                                      kind="ExternalOutput"),
            "dbg_ol7": nc.dram_tensor("dbg_ol7", [128, SH],
                                      mybir.dt.bfloat16,
                                      kind="ExternalOutput"),
        }

    PS = bass.MemorySpace.PSUM

    with tile.TileContext(nc) as tc, ExitStack() as ctx:
        # ---- constants (live whole kernel) -------------------------------
        cpool = ctx.enter_context(tc.tile_pool(name="consts", bufs=1))
        ones_bf = cpool.tile([1, 128], bf16, tag="ones", name="ones")
        nc.vector.memset(ones_bf[:], 1.0)
        eps_col = cpool.tile([128, 1], f32, tag="eps", name="eps")
        nc.vector.memset(eps_col[:], float(LN_EPS))
        brow_sb = {}
        for nm, d in (("bv_g", bv_g_d), ("bv_l", bv_l_d), ("bo", bo_d)):
            t = cpool.tile([1, d.shape[1]], bf16, tag=nm, name=nm)
            nc.sync.dma_start(t[:], d[:])
            brow_sb[nm] = t
        bcol_sb = {}
        for nm, d in bcol_d.items():
            cols = []
            for j in range(NHK):
                t = cpool.tile([128, 1], f32, tag=f"{nm}{j}", name=f"{nm}{j}")
                nc.sync.dma_start(t[:], d[j, :].rearrange("(a b) -> a b", b=1))
                cols.append(t)
            bcol_sb[nm] = cols

        # x^T resident in SBUF for all projections
        xpool = ctx.enter_context(tc.tile_pool(name="xin", bufs=1))
        xTp = [xpool.tile([128, 2, S], fp8, tag=f"xt{dp}", name=f"xt{dp}")
               for dp in range(ND // 2)]
        for dp in range(ND // 2):
            for j in range(2):
                nc.sync.dma_start(
                    xTp[dp][:, j, :],
                    xT_d[(2 * dp + j) * 128:(2 * dp + j + 1) * 128, :])

        # o accumulators for out-projection (all head-pairs, both sets)
        opool = ctx.enter_context(tc.tile_pool(name="oacc", bufs=1))
        o_sb = {st: [opool.tile([128, SH], bf16, tag=f"ob{st}{t}",
                                name=f"ob{st}{t}")
                     for t in range(NHK)] for st in ("g", "l")}

        # wo prefetched early (overlaps the attention pipeline); in bf16
        # bisect mode SBUF is tight, so wo loads late instead
        wo_sb = {}

        def load_wo(pool):
            for st_, d in (("g", wo_g_d), ("l", wo_l_d)):
                wo_sb[st_] = [pool.tile([128, D], bf16, tag=f"wo{st_}{t}",
                                        name=f"wo{st_}{t}")
                              for t in range(NHK)]
                for t in range(NHK):
                    nc.sync.dma_start(wo_sb[st_][t][:],
                                      d[t * 128:(t + 1) * 128, :])
        wop = ctx.enter_context(tc.tile_pool(name="wo", bufs=1))

        # final-phase tiles that must coexist with the attention pools
        fin = ctx.enter_context(tc.tile_pool(name="fin", bufs=1))
        gamma_bc = fin.tile([128, D], bf16, tag="gamma", name="gamma",
                            bufs=1)
        nc.sync.dma_start(gamma_bc[:], gamma_d[:].partition_broadcast(128))
        beta_bc = fin.tile([128, D], bf16, tag="beta", name="beta", bufs=1)
        nc.sync.dma_start(beta_bc[:], beta_d[:].partition_broadcast(128))
        y_tiles = {}

        # ---- per-head-pair pools (double buffered across hp) -------------
        hp_ctx = ExitStack()
        kqv = hp_ctx.enter_context(tc.tile_pool(name="kqv", bufs=2))
        wts = hp_ctx.enter_context(tc.tile_pool(name="wts", bufs=2))
        vxp = hp_ctx.enter_context(tc.tile_pool(name="vxp", bufs=2))
        ppsum = hp_ctx.enter_context(
            tc.tile_pool(name="ppsum", bufs=2, space=PS))
        scp = hp_ctx.enter_context(tc.tile_pool(name="scp", bufs=2, space=PS))
        ovp = hp_ctx.enter_context(tc.tile_pool(name="ovp", bufs=1, space=PS))
        exp_p = hp_ctx.enter_context(
            tc.tile_pool(name="exp", bufs=3 if av_mode == "dr_fp8" else 2))
        nop = hp_ctx.enter_context(tc.tile_pool(name="norm", bufs=1))

        def load_kq_weights(hp):
            out = {}
            for nm in ("wk_g", "wq_g", "wk_l", "wq_l"):
                t = wts.tile([128, ND * 128], fp8, tag=nm, name=nm)
                for d in range(ND):
                    nc.sync.dma_start(t[:, d * 128:(d + 1) * 128],
                                      w_d[nm][hp, d])
                out[nm] = t
            return out

        def load_v_weights(grp):
            out = {}
            for nm in ("wv_g", "wv_l"):
                t = wts.tile([128, ND, GHP * 128], fp8, tag=f"{nm}4",
                             name=f"{nm}4", bufs=1)
                for d in range(ND):
                    nc.sync.dma_start(t[:, d, :], wv_d[nm][grp, d])
                out[nm] = t
            return out

        def vproj_gen(grp, wv, dst):
            """V projections for head-pair group grp (4 head-pairs), fp8
            DoubleRow layout. dst['vg'/'vl'] maps hp -> list of per-pair
            tiles [128, 2(sub), 2(j), 80]."""
            # group broadcast of bv ([1, 512] -> [128, 512])
            bvbc = {}
            for nm, brow in (("wv_g", "bv_g"), ("wv_l", "bv_l")):
                b = nop.tile([128, GHP * 128], bf16, tag=f"bv{nm}",
                             name=f"bv{nm}")
                nc.gpsimd.partition_broadcast(
                    b[:], brow_sb[brow][0:1,
                                        grp * GHP * 128:(grp + 1) * GHP * 128])
                bvbc[nm] = b
            for nm, n_t, key in (("wv_g", NST, "vg"), ("wv_l", NQT, "vl")):
                tiles = {}
                for i in range(GHP):
                    tiles[grp * GHP + i] = dst[key][grp * GHP + i] = []
                for t in range(n_t):
                    pt = ppsum.tile([128, 512], f32, tag="pp", name="pp")
                    for dp in range(ND // 2):
                        nc.tensor.matmul(
                            pt[:, 0:GHP * 128],
                            xTp[dp][:, :, t * 128:(t + 1) * 128],
                            wv[nm][:].rearrange(
                                "p (dp j) c -> p dp j c", j=2)[:, dp],
                            start=(dp == 0), stop=(dp == ND // 2 - 1),
                            perf_mode=DR)
                    for i in range(GHP):
                        hp = grp * GHP + i
                        if t % 2 == 0:
                            vt = vxp.tile([128, 2, 2, 80], fp8,
                                          tag=f"{key}{i}{t // 2}",
                                          name=f"{key}{i}{t // 2}")
                            tiles[hp].append(vt)
                            for s_ in range(2):
                                nc.vector.memset(vt[:, s_, :, 64:80], 0.0)
                                nc.vector.memset(vt[:, s_, :, 64:65], 1.0)
                        vt = tiles[hp][t // 2]
                        nc.vector.tensor_tensor(
                            vt[:, :, t % 2, 0:64],
                            pt[:, i * 128:(i + 1) * 128].rearrange(
                                "p (h k) -> p h k", k=64),
                            bvbc[nm][:, i * 128:(i + 1) * 128].rearrange(
                                "p (h k) -> p h k", k=64),
                            add_op)
                    if t % 2 == 1:
                        yield

        def proj_gen(hp, w, dst):
            """kq projections for head-pair hp: out[hkp, s] = (x@w)^T + b."""
            for nm, s_len, key in (("wk_g", S, "kg"), ("wq_g", SH, "qg"),
                                   ("wk_l", SH, "kl"), ("wq_l", SH, "ql")):
                ot = kqv.tile([128, s_len], bf16, tag=key, name=key)
                dst[key] = ot
                bias = bcol_sb["b" + nm[1:]][hp]
                for so, sl in _chunks(s_len, 512):
                    pt = ppsum.tile([128, 512], f32, tag="pp", name="pp")
                    for dp in range(ND // 2):
                        nc.tensor.matmul(
                            pt[:, 0:sl],
                            w[nm][:].rearrange(
                                "p (dp j c) -> p dp j c", j=2, c=128)[:, dp],
                            xTp[dp][:, :, so:so + sl],
                            start=(dp == 0), stop=(dp == ND // 2 - 1),
                            perf_mode=DR)
                    nc.vector.tensor_scalar(
                        ot[:, so:so + sl], pt[:, 0:sl], bias, None, add_op)
                    yield

        def normalize(o_ps, qo, dst_tiles, hp):
            """Raw-evac o_ps pair (2 heads x [65, QC]) then normalize into
            dst_tiles[hp] columns [qo:qo+QC]."""
            for sub in range(2):
                rs = nop.tile([1, QC], f32, tag=f"rs{sub}", name=f"rs{sub}")
                nc.vector.tensor_copy(rs[:], o_ps[sub][64:65, :])
                rinv = nop.tile([1, QC], f32, tag=f"ri{sub}", name=f"ri{sub}")
                # custom-DVE op: input must be a base-partition-0 SBUF AP
                nc.vector.reciprocal_approx_fast(rinv[:], rs[:])
                rb = nop.tile([64, QC], f32, tag=f"rb{sub}", name=f"rb{sub}")
                nc.gpsimd.partition_broadcast(rb[:], rinv[0:1, :])
                nc.vector.tensor_tensor(
                    dst_tiles[hp][sub * 64:sub * 64 + 64, qo:qo + QC],
                    o_ps[sub][0:64, :], rb[:], mult_op)

        def attn_gen(hp, src):
            """Attention (global + local) for head-pair hp."""
            kg, qg = src["kg"], src["qg"]
            kl, ql_ = src["kl"], src["ql"]
            vg, vl = vkeys["vg"][hp], vkeys["vl"][hp]
            # ---- global: q-chunks outer, s-tile-pairs inner --------------
            for qc in range(NQC):
                qo = qc * QC
                o_ps = [ovp.tile([65, QC], f32, tag=f"o{sub}", name=f"o{sub}")
                        for sub in range(2)]
                for tp in range(NST // 2):
                    ex = exp_p.tile([128, 2, 2, QC], fp8, tag="ex", name="ex")
                    for j in range(2):
                        t = 2 * tp + j
                        sc = scp.tile([128, 2, QC], f32, tag="sc", name="sc")
                        for sub in range(2):
                            po = sub * 64
                            nc.tensor.matmul(
                                sc[:, sub, :],
                                kg[po:po + 64, t * 128:(t + 1) * 128],
                                qg[po:po + 64, qo:qo + QC],
                                start=True, stop=True)
                        nc.scalar.activation(ex[:, j], sc[:], Exp,
                                             scale=0.125)
                    for sub in range(2):
                        if av_mode == "dr_fp8":
                            nc.tensor.matmul(
                                o_ps[sub][:],
                                vg[tp][:, sub, :, 0:65],
                                ex[:, :, sub, :],
                                start=(tp == 0), stop=(tp == NST // 2 - 1),
                                perf_mode=DR)
                        else:
                            for j in range(2):
                                nc.tensor.matmul(
                                    o_ps[sub][:],
                                    vg[tp][:, sub, j, 0:65],
                                    ex[:, j, sub, :],
                                    start=(tp == 0 and j == 0),
                                    stop=(tp == NST // 2 - 1 and j == 1))
                    yield
                normalize(o_ps, qo, o_sb["g"], hp)
                yield
            # ---- local: q-chunk = window pair, DoubleRow over ss ---------
            for wp in range(NWH // 2):
                qo = wp * QC
                o_ps = [ovp.tile([65, QC], f32, tag=f"o{sub}", name=f"o{sub}")
                        for sub in range(2)]
                ex = exp_p.tile([128, 2, 2, QC], fp8, tag="ex", name="ex")
                for ss in range(2):
                    sc = scp.tile([128, 2, QC], f32, tag="sc", name="sc")
                    for sub in range(2):
                        po = sub * 64
                        for wi in range(2):
                            w = 2 * wp + wi
                            st_ = 2 * w + ss
                            nc.tensor.matmul(
                                sc[:, sub, wi * 256:wi * 256 + 256],
                                kl[po:po + 64, st_ * 128:(st_ + 1) * 128],
                                ql_[po:po + 64,
                                    qo + wi * 256:qo + wi * 256 + 256],
                                start=(wi == 0), stop=(wi == 1))
                    nc.scalar.activation(ex[:, ss], sc[:], Exp,
                                         scale=0.125)
                for sub in range(2):
                    for wi in range(2):
                        w = 2 * wp + wi
                        if av_mode == "dr_fp8":
                            nc.tensor.matmul(
                                o_ps[sub][:, wi * 256:wi * 256 + 256],
                                vl[w][:, sub, :, 0:65],
                                ex[:, :, sub, wi * 256:wi * 256 + 256],
                                start=(wi == 0), stop=(wi == 1),
                                perf_mode=DR)
                        else:
                            for ss in range(2):
                                nc.tensor.matmul(
                                    o_ps[sub][:, wi * 256:wi * 256 + 256],
                                    vl[w][:, sub, ss, 0:65],
                                    ex[:, ss, sub, wi * 256:wi * 256 + 256],
                                    start=(wi == 0 and ss == 0),
                                    stop=(wi == 1 and ss == 1))
                yield
                normalize(o_ps, qo, o_sb["l"], hp)
                yield

        def outproj_partial_gen():
            """Out-projection partial sums (head-pairs 0..NHK-2) + residual,
            overlapped with the last attention; leaves y = x + partial."""
            for qt in range(NQT):
                xq_t = fin.tile([128, D], f32, tag="xq", name="xq", bufs=2)
                nc.sync.dma_start(xq_t[:], xq_d[qt * 128:(qt + 1) * 128, :])
                y = fin.tile([128, D], bf16, tag="y", name="y", bufs=NQT)
                y_tiles[qt] = y
                for do, dl in _chunks(D, 512):
                    pt = ppsum.tile([128, 512], f32, tag="pp", name="pp")
                    first = True
                    for st_ in ("g", "l"):
                        for t in range(NHK - 1):
                            nc.tensor.matmul(
                                pt[:, 0:dl],
                                o_sb[st_][t][:, qt * 128:(qt + 1) * 128],
                                wo_sb[st_][t][:, do:do + dl],
                                start=first,
                                stop=(st_ == "l" and t == NHK - 2))
                            first = False
                    nc.vector.tensor_tensor(
                        y[:, do:do + dl], pt[:, 0:dl],
                        xq_t[:, do:do + dl], add_op)
                    yield

        # ---- software-pipelined main loop over head-pairs ----------------
        import itertools
        kq_w = load_kq_weights(0)
        v_w = load_v_weights(0)
        src = {}
        prev_src = None
        vkeys = {"vg": {}, "vl": {}}
        for hp in range(NHK):
            if hp == 0:
                pg = itertools.chain(proj_gen(hp, kq_w, src),
                                     vproj_gen(0, v_w, vkeys))
            elif hp % GHP == 0:
                pg = itertools.chain(vproj_gen(hp // GHP, v_w, vkeys),
                                     proj_gen(hp, kq_w, src))
            else:
                pg = proj_gen(hp, kq_w, src)
            ag = attn_gen(hp - 1, prev_src) if prev_src is not None else None
            if hp == 2:
                load_wo(wop)
            if hp + 1 < NHK:
                kq_next = load_kq_weights(hp + 1)
            if hp % GHP == 1 and hp + GHP < NHK + 1:
                v_next = load_v_weights((hp + GHP) // GHP)
            # interleave: 2 attention steps per projection step
            done_p, done_a = False, ag is None
            while not (done_p and done_a):
                if not done_a:
                    done_a = next(ag, "END") == "END"
                if not done_p:
                    done_p = next(pg, "END") == "END"
                if not done_a:
                    done_a = next(ag, "END") == "END"
            if hp + 1 < NHK:
                kq_w = kq_next
            if hp % GHP == 1 and hp + GHP < NHK + 1:
                v_w = v_next
            prev_src, src = src, {}
        ag = attn_gen(NHK - 1, prev_src)
        pg = outproj_partial_gen()
        done_p = done_a = False
        while not (done_p and done_a):
            if not done_a:
                done_a = next(ag, "END") == "END"
            if not done_p:
                done_p = next(pg, "END") == "END"
            if not done_a:
                done_a = next(ag, "END") == "END"
        if dbg:
            nc.sync.dma_start(dbg_d["dbg_kg"][:], prev_src["kg"][:])
            nc.sync.dma_start(dbg_d["dbg_qg"][:], prev_src["qg"][:])
            nc.sync.dma_start(
                dbg_d["dbg_vg"][:],
                vkeys["vg"][NHK - 1][0][:].rearrange("p a b c -> p (a b c)"))
            nc.sync.dma_start(dbg_d["dbg_og0"][:], o_sb["g"][0][:])
            nc.sync.dma_start(dbg_d["dbg_og7"][:], o_sb["g"][NHK - 1][:])
            nc.sync.dma_start(dbg_d["dbg_ol7"][:], o_sb["l"][NHK - 1][:])
        hp_ctx.close()

        # ---- finish: last head-pair + bias, then layernorm ---------------
        with tc.tile_pool(name="ypsum", bufs=2, space=PS) as ypp, \
             tc.tile_pool(name="ln", bufs=2) as lnp:
            for qt in range(NQT):
                y = y_tiles[qt]
                for do, dl in _chunks(D, 512):
                    ps_y = ypp.tile([128, 512], f32, tag="py", name="py")
                    for st_ in ("g", "l"):
                        nc.tensor.matmul(
                            ps_y[:, 0:dl],
                            o_sb[st_][NHK - 1][:, qt * 128:(qt + 1) * 128],
                            wo_sb[st_][NHK - 1][:, do:do + dl],
                            start=(st_ == "g"), stop=False)
                    nc.tensor.matmul(
                        ps_y[:, 0:dl], ones_bf[0:1, 0:128],
                        brow_sb["bo"][0:1, do:do + dl], start=False, stop=True)
                    nc.vector.tensor_tensor(y[:, do:do + dl], y[:, do:do + dl],
                                            ps_y[:, 0:dl], add_op)
                ssum = lnp.tile([128, 1], f32, tag="ssum", name="ssum")
                nc.vector.reduce_sum(ssum[:], y[:], axis=AxX)
                sqd = lnp.tile([128, D], bf16, tag="sqd", name="sqd")
                ssq = lnp.tile([128, 1], f32, tag="ssq", name="ssq")
                nc.scalar.activation(sqd[:], y[:], Square, accum_out=ssq[:])
                mu = lnp.tile([128, 1], f32, tag="mu", name="mu")
                nc.vector.tensor_scalar_mul(mu[:], ssum[:], 1.0 / D)
                var = lnp.tile([128, 1], f32, tag="var", name="var")
                nc.vector.tensor_scalar_mul(var[:], ssq[:], 1.0 / D)
                mu2 = lnp.tile([128, 1], f32, tag="mu2", name="mu2")
                nc.vector.tensor_tensor(mu2[:], mu[:], mu[:], mult_op)
                nc.vector.tensor_tensor(var[:], var[:], mu2[:], sub_op)
                sd = lnp.tile([128, 1], f32, tag="sd", name="sd")
                nc.scalar.activation(sd[:], var[:], Sqrt, bias=eps_col[:])
                rstd = lnp.tile([128, 1], f32, tag="rstd", name="rstd")
                nc.vector.reciprocal(rstd[:], sd[:])
                bco = lnp.tile([128, 1], f32, tag="bco", name="bco")
                nc.vector.tensor_tensor(bco[:], mu[:], rstd[:], mult_op)
                nc.vector.tensor_scalar_mul(bco[:], bco[:], -1.0)
                t1 = lnp.tile([128, D], f32, tag="t1", name="t1")
                nc.vector.tensor_scalar(t1[:], y[:], rstd[:], bco[:],
                                        mult_op, add_op)
                t2 = lnp.tile([128, D], f32, tag="t2", name="t2")
                nc.vector.tensor_tensor(t2[:], t1[:], gamma_bc[:], mult_op)
                ot = lnp.tile([128, D], f32, tag="ot", name="ot")
                nc.vector.tensor_tensor(ot[:], t2[:], beta_bc[:], add_op)
                nc.sync.dma_start(out_d[qt * 128:(qt + 1) * 128, :], ot[:])

    nc.compile()
    return nc


def make_in_maps(inputs, cfg=None):
    """Build per-core input maps from the full (unsharded) problem inputs."""
    cfg = dict(cfg or FULL_CFG)
    S, D, H, K = cfg["S"], cfg["D"], cfg["H"], cfg["K"]
    HK = H * K
    SH = S // 2
    NHK = HK // 128
    ND = D // 128
    NGRP = 2
    GW = HK // NGRP

    def np32(a):
        return np.asarray(a, dtype=np.float32)

    shared = {}
    for nm, key in (("wq_g", "gWq"), ("wk_g", "gWk"),
                    ("wq_l", "lWq"), ("wk_l", "lWk")):
        w = np32(inputs[key]).reshape(D, HK)
        shared[nm] = np.ascontiguousarray(
            w.reshape(ND, 128, NHK, 128).transpose(2, 0, 1, 3)).astype(FP8)
    for nm, key in (("wv_g", "gWv"), ("wv_l", "lWv")):
        w = np32(inputs[key]).reshape(D, HK)
        shared[nm] = np.ascontiguousarray(
            w.reshape(ND, 128, NGRP, GW).transpose(2, 0, 1, 3)).astype(FP8)
    shared["wo_g"] = np.ascontiguousarray(
        np32(inputs["gWo"]).reshape(HK, D)).astype(BF16)
    shared["wo_l"] = np.ascontiguousarray(
        np32(inputs["lWo"]).reshape(HK, D)).astype(BF16)
    for nm, key in (("bq_g", "gbq"), ("bk_g", "gbk"),
                    ("bq_l", "lbq"), ("bk_l", "lbk")):
        shared[nm] = np.ascontiguousarray(np32(inputs[key]).reshape(NHK, 128))
    shared["bv_g"] = np32(inputs["gbv"]).reshape(1, HK).astype(BF16)
    shared["bv_l"] = np32(inputs["lbv"]).reshape(1, HK).astype(BF16)
    shared["bo"] = (np32(inputs["gbo"]) +
                    np32(inputs["lbo"])).reshape(1, D).astype(BF16)
    shared["gamma"] = np32(inputs["gamma"]).reshape(1, D).astype(BF16)
    shared["beta"] = np32(inputs["beta"]).reshape(1, D).astype(BF16)

    x = np32(inputs["x"])
    in_maps = []
    for c in range(N_CORES):
        b, half = divmod(c, 2)
        xb = x[b]
        # own half first (queries/local), other half second; global attention
        # is invariant to key/value column order
        xperm = np.concatenate([xb[half * SH:(half + 1) * SH],
                                xb[(1 - half) * SH:(2 - half) * SH]], axis=0)
        m = dict(shared)
        m["xT"] = np.ascontiguousarray(xperm.T).astype(FP8)
        m["xq"] = np.ascontiguousarray(xperm[0:SH])
        in_maps.append(m)
    return in_maps


def assemble_out(results, cfg=None):
    cfg = dict(cfg or FULL_CFG)
    S, D = cfg["S"], cfg["D"]
    SH = S // 2
    B = N_CORES // 2
    out = np.empty((B, S, D), np.float32)
    for c in range(N_CORES):
        b, half = divmod(c, 2)
        out[b, half * SH:(half + 1) * SH] = results[c]["out"]
    return out


_NC_CACHE = {}


def kernel(**inputs):
    from concourse.bass_utils import run_bass_kernel_spmd
    if "nc" not in _NC_CACHE:
        _NC_CACHE["nc"] = build_nc()
    nc = _NC_CACHE["nc"]
    in_maps = make_in_maps(inputs)
    res = run_bass_kernel_spmd(nc, in_maps, list(range(N_CORES)))
    return assemble_out(res.results)

